# revision 3
# baseline (speedup 1.0000x reference)
"""Trainium2 Bass kernel for nn_Encoder_74947179316057.

Reference computation: pack hist_traj [64,128,3] to 8192 tokens, run each
token vector through three stride-2 ConvTranspose2d+ReLU layers
(1x1 -> 3x3 -> 7x7 -> 16x16), then the reference's un-slicing makes the
output depend only on the first 2*T tokens:
  enc_out[0:T]   = f(tokens of batch 0)           (block A)
  enc_out[T:B*T] = f(tokens of batch 1) tiled 63x (block B)

Kernel strategy (8 cores, data-parallel over output rows):
  core k owns output rows [k*1024, (k+1)*1024) = 8 row-groups of T=128.
  Every core computes f() for two 128-token blocks (xa, xb) fully on-chip
  and writes its 64MB output shard; block-B results are written to 7
  row-group destinations straight out of SBUF, so HBM traffic is just the
  output (memory-bound regime).

Each ConvTranspose2d(stride 2, k=3) is decomposed by output-pixel parity
class (y%2, x%2): class (ry,rx) output [Cout, t, nu, nv] is a PSUM-accumulated
sum over kernel taps (ky,kx) with ky=ry or ry+2, kx=rx or rx+2, each tap a
matmul of w[:, :, ky, kx] (lhsT [Cin,Cout]) against a shifted window of the
zero-padded previous activation (rhs [Cin, t*nu*nv]).  ReLU+bias happen in
the PSUM->SBUF scatter (scalar engine activation with per-partition bias).
"""

import os
import sys

import numpy as np

for _p in ("/opt/trn_rl_repo", "/root/.axon_site/_ro/trn_rl_repo"):
    if os.path.isdir(_p) and _p not in sys.path:
        sys.path.append(_p)

B, T, C = 64, 128, 3
N_CORES = 8
REPS = (B * T) // N_CORES // T  # 8 row-groups of T rows per core
CHUNK = 16   # tokens per L3 output chunk (SBUF staging tile)
TG2 = 32     # tokens per L2 psum group (32*16 pix = 512 max free)
TG3 = 8      # tokens per L3 psum group (8*64 pix = 512 free)

_cache = {}


def _class_taps(r):
    """Kernel taps contributing to output parity class r (stride 2, k=3)."""
    return (r, r + 2) if r == 0 else (r,)


def _build_phase(nc, tc, ctx, mybir, xs, w1s, w2s, w3s, b1s, b2s, b3s,
                 pools, out_v, phase):
    f32 = mybir.dt.float32
    Relu = mybir.ActivationFunctionType.Relu
    a1p, a2p, ocp, p1p, p2p, p3p = pools

    # ---- L1: token vec [3] -> a1 [16, t, 3, 3], stored zero-padded 5x5.
    a1 = a1p.tile([16, T * 25], f32, tag="a1")
    nc.vector.memset(a1[:], 0.0)
    a1v = a1[:].rearrange("p (t i j) -> p t i j", t=T, i=5, j=5)
    for p_ in range(3):
        for q_ in range(3):
            pos = p_ * 3 + q_
            ps = p1p.tile([16, T], f32, tag="p1")
            nc.tensor.matmul(ps[:], w1s[:][:, pos * 16:(pos + 1) * 16],
                             xs[:], start=True, stop=True)
            nc.scalar.activation(a1v[:, :, 1 + p_, 1 + q_], ps[:], Relu,
                                 bias=b1s[:])

    # ---- L2: a1 [16, t, 3, 3] -> a2 [32, t, 7, 7], stored zero-padded 9x9.
    a2 = a2p.tile([32, T * 81], f32, tag="a2")
    nc.gpsimd.memset(a2[:], 0.0)
    a2v = a2[:].rearrange("p (t i j) -> p t i j", t=T, i=9, j=9)
    for ry in (0, 1):
        nu = 4 if ry == 0 else 3
        for rx in (0, 1):
            nv = 4 if rx == 0 else 3
            taps = [(ky, kx) for ky in _class_taps(ry) for kx in _class_taps(rx)]
            for t0 in range(0, T, TG2):
                ps = p2p.tile([32, TG2 * nu * nv], f32, tag="p2")
                for i, (ky, kx) in enumerate(taps):
                    pp0 = 1 if ky == ry else 0
                    qq0 = 1 if kx == rx else 0
                    rhs = a1v[:, t0:t0 + TG2, pp0:pp0 + nu, qq0:qq0 + nv]
                    tap = ky * 3 + kx
                    nc.tensor.matmul(ps[:], w2s[:][:, tap * 32:(tap + 1) * 32],
                                     rhs, start=(i == 0),
                                     stop=(i == len(taps) - 1))
                psv = ps[:].rearrange("p (t u v) -> p t u v",
                                      t=TG2, u=nu, v=nv)
                dest = a2v[:, t0:t0 + TG2,
                           1 + ry:1 + ry + 2 * nu:2,
                           1 + rx:1 + rx + 2 * nv:2]
                nc.scalar.activation(dest, psv, Relu, bias=b2s[:])

    # ---- L3: a2 [32, t, 7, 7] -> out [64, t, 16, 16], chunked by CHUNK
    # tokens; each chunk DMA'd to its output row-group destination(s).
    for c0 in range(0, T, CHUNK):
        oc = ocp.tile([64, CHUNK * 256], f32, tag="oc")
        ocv = oc[:].rearrange("p (t y x) -> p t y x", t=CHUNK, y=16, x=16)
        for ry in (0, 1):
            for rx in (0, 1):
                taps = [(ky, kx) for ky in _class_taps(ry)
                        for kx in _class_taps(rx)]
                for t0 in range(c0, c0 + CHUNK, TG3):
                    ps = p3p.tile([64, TG3 * 64], f32, tag="p3")
                    for i, (ky, kx) in enumerate(taps):
                        pp0 = 1 if ky == ry else 0
                        qq0 = 1 if kx == rx else 0
                        rhs = a2v[:, t0:t0 + TG3, pp0:pp0 + 8, qq0:qq0 + 8]
                        tap = ky * 3 + kx
                        nc.tensor.matmul(ps[:],
                                         w3s[:][:, tap * 64:(tap + 1) * 64],
                                         rhs, start=(i == 0),
                                         stop=(i == len(taps) - 1))
                    psv = ps[:].rearrange("p (t u v) -> p t u v",
                                          t=TG3, u=8, v=8)
                    dest = ocv[:, t0 - c0:t0 - c0 + TG3, ry::2, rx::2]
                    nc.scalar.activation(dest, psv, Relu, bias=b3s[:])
        ocs = oc[:].rearrange("p (t yx) -> p t yx", t=CHUNK, yx=256)
        reps = (0,) if phase == 0 else tuple(range(1, REPS))
        for rep in reps:
            row0 = rep * T + c0
            nc.sync.dma_start(out_v[:, row0:row0 + CHUNK, :], ocs)


def _build(repeats=1):
    from contextlib import ExitStack

    import concourse.bacc as bacc
    import concourse.tile as tile
    from concourse import mybir

    f32 = mybir.dt.float32
    nc = bacc.Bacc("TRN2", target_bir_lowering=False, debug=False)
    xa = nc.dram_tensor("xa", [3, T], f32, kind="ExternalInput").ap()
    xb = nc.dram_tensor("xb", [3, T], f32, kind="ExternalInput").ap()
    w1d = nc.dram_tensor("w1r", [3, 144], f32, kind="ExternalInput").ap()
    w2d = nc.dram_tensor("w2r", [16, 288], f32, kind="ExternalInput").ap()
    w3d = nc.dram_tensor("w3r", [32, 576], f32, kind="ExternalInput").ap()
    b1d = nc.dram_tensor("b1c", [16, 1], f32, kind="ExternalInput").ap()
    b2d = nc.dram_tensor("b2c", [32, 1], f32, kind="ExternalInput").ap()
    b3d = nc.dram_tensor("b3c", [64, 1], f32, kind="ExternalInput").ap()
    out = nc.dram_tensor("out", [REPS * T, 64 * 256], f32,
                         kind="ExternalOutput").ap()
    out_v = out.rearrange("r (o yx) -> o r yx", o=64, yx=256)

    with tile.TileContext(nc) as tc:
        with ExitStack() as ctx:
            const = ctx.enter_context(tc.tile_pool(name="const", bufs=1))
            a1p = ctx.enter_context(tc.tile_pool(name="a1", bufs=2))
            a2p = ctx.enter_context(tc.tile_pool(name="a2", bufs=2))
            ocp = ctx.enter_context(tc.tile_pool(name="oc", bufs=3))
            p1p = ctx.enter_context(tc.tile_pool(name="p1", bufs=2,
                                                 space="PSUM"))
            p2p = ctx.enter_context(tc.tile_pool(name="p2", bufs=2,
                                                 space="PSUM"))
            p3p = ctx.enter_context(tc.tile_pool(name="p3", bufs=4,
                                                 space="PSUM"))
            pools = (a1p, a2p, ocp, p1p, p2p, p3p)

            def load_const(shape, src, tag):
                t = const.tile(shape, f32, tag=tag)
                nc.sync.dma_start(t[:], src)
                return t

            w1s = load_const([3, 144], w1d, "w1")
            w2s = load_const([16, 288], w2d, "w2")
            w3s = load_const([32, 576], w3d, "w3")
            b1s = load_const([16, 1], b1d, "b1")
            b2s = load_const([32, 1], b2d, "b2")
            b3s = load_const([64, 1], b3d, "b3")
            xas = load_const([3, T], xa, "xa")
            xbs = load_const([3, T], xb, "xb")

            for _rep in range(repeats):
                for phase, xs in ((0, xas), (1, xbs)):
                    _build_phase(nc, tc, ctx, mybir, xs, w1s, w2s, w3s,
                                 b1s, b2s, b3s, pools, out_v, phase)
    nc.compile()
    return nc


def _get_nc():
    if "nc" not in _cache:
        _cache["nc"] = _build()
    return _cache["nc"]


def _prep_in_maps(hist_traj, w1, b1, w2, b2, w3, b3):
    h = np.where(hist_traj == -1.0, 0.0, hist_traj).astype(np.float32)
    xA = np.ascontiguousarray(h[0].T)  # [3, T]
    xB = np.ascontiguousarray(h[1].T)
    common = {
        "w1r": np.ascontiguousarray(
            w1.transpose(0, 2, 3, 1).reshape(3, 144)).astype(np.float32),
        "w2r": np.ascontiguousarray(
            w2.transpose(0, 2, 3, 1).reshape(16, 288)).astype(np.float32),
        "w3r": np.ascontiguousarray(
            w3.transpose(0, 2, 3, 1).reshape(32, 576)).astype(np.float32),
        "b1c": np.ascontiguousarray(b1.reshape(16, 1)).astype(np.float32),
        "b2c": np.ascontiguousarray(b2.reshape(32, 1)).astype(np.float32),
        "b3c": np.ascontiguousarray(b3.reshape(64, 1)).astype(np.float32),
        "xb": xB,
    }
    in_maps = []
    for core in range(N_CORES):
        m = dict(common)
        m["xa"] = xA if core == 0 else xB
        in_maps.append(m)
    return in_maps


def run_on_hw(hist_traj, w1, b1, w2, b2, w3, b3, trace=False,
              trace_cores=None):
    """Returns ((enc_out, seq_len), BassKernelResults)."""
    from concourse.bass_utils import run_bass_kernel_spmd

    nc = _get_nc()
    in_maps = _prep_in_maps(hist_traj, w1, b1, w2, b2, w3, b3)
    res = run_bass_kernel_spmd(nc, in_maps, list(range(N_CORES)),
                               trace=trace, trace_cores=trace_cores)
    enc = np.concatenate([r["out"] for r in res.results], axis=0)
    enc = enc.reshape(B * T, 64, 16, 16)
    seq_len = np.full((B,), T, dtype=np.int32)
    return (enc, seq_len), res


def kernel(hist_traj, w1, b1, w2, b2, w3, b3):
    out, _ = run_on_hw(hist_traj, w1, b1, w2, b2, w3, b3)
    return out


# revision 17
# speedup vs baseline: 1.4566x; 1.4566x over previous
"""Trainium2 Bass kernel for nn_Encoder_74947179316057.

Reference computation: pack hist_traj [64,128,3] to 8192 tokens, run each
token vector through three stride-2 ConvTranspose2d+ReLU layers
(1x1 -> 3x3 -> 7x7 -> 16x16), then the reference's un-slicing makes the
output depend only on the first 2*T tokens:
  enc_out[0:T]   = f(tokens of batch 0)           (block A)
  enc_out[T:B*T] = f(tokens of batch 1) tiled 63x (block B)

Kernel strategy (8 cores, data-parallel over output rows):
  core k owns output rows [k*1024, (k+1)*1024) = 8 row-groups of T=128.
  Every core computes f() for two 128-token blocks (xa, xb) fully on-chip
  and writes its 64MB output shard; block-B results are written to 7
  row-group destinations straight out of SBUF, so HBM traffic is just the
  output (memory-bound regime).

Each ConvTranspose2d(stride 2, k=3) is decomposed by output-pixel parity
class (y%2, x%2): class (ry,rx) output [Cout, t, nu, nv] is a PSUM-accumulated
sum over kernel taps (ky,kx) with ky=ry or ry+2, kx=rx or rx+2, each tap a
matmul of w[:, :, ky, kx] (lhsT [Cin,Cout]) against a shifted window of the
zero-padded previous activation (rhs [Cin, t*nu*nv]).  ReLU+bias happen in
the PSUM->SBUF scatter (scalar engine activation with per-partition bias).
"""

import os
import sys

import numpy as np

for _p in ("/opt/trn_rl_repo", "/root/.axon_site/_ro/trn_rl_repo"):
    if os.path.isdir(_p) and _p not in sys.path:
        sys.path.append(_p)

B, T, C = 64, 128, 3
N_CORES = 8
REPS = (B * T) // N_CORES // T  # 8 row-groups of T rows per core
CHUNK = 16   # tokens per L3 output chunk (SBUF staging tile)
TG2 = 32     # tokens per L2 psum group (32*16 pix = 512 max free)
TG3 = 8      # tokens per L3 psum group (8*64 pix = 512 free)

_cache = {}

# Matmul input dtype: float32r streams fp32 data faster through the PE
# but is reduced precision and needs an explicit rounding producer
# (walrus BIR verifier rejects plain-fp32-fed fp32r matmuls).  Keep
# exact fp32; PE cost is managed by K-stacking taps instead.
MM_F32R = False


def _class_taps(r):
    """Kernel taps contributing to output parity class r (stride 2, k=3)."""
    return (r, r + 2) if r == 0 else (r,)


def _build_phase(nc, tc, ctx, mybir, xs, w1s, w2s, w3s, b1s, b2s, b3s,
                 pools, out_v, phase):
    f32 = mybir.dt.float32
    Relu = mybir.ActivationFunctionType.Relu
    a1p, a2p, ocp, p1p, p2p, p3p = pools

    def mm(out, lhsT, rhs, start, stop):
        if MM_F32R:
            lhsT = lhsT.bitcast(mybir.dt.float32r)
            rhs = rhs.bitcast(mybir.dt.float32r)
        nc.tensor.matmul(out, lhsT, rhs, start=start, stop=stop)

    # ---- L1: token vec [3] -> a1 [16, t, 3, 3], stored zero-padded 5x5.
    a1 = a1p.tile([16, T * 25], f32, tag="a1")
    nc.vector.memset(a1[:], 0.0)
    a1v = a1[:].rearrange("p (t i j) -> p t i j", t=T, i=5, j=5)
    for p_ in range(3):
        for q_ in range(3):
            pos = p_ * 3 + q_
            ps = p1p.tile([16, T], f32, tag="p1")
            mm(ps[:], w1s[:][:, pos * 16:(pos + 1) * 16],
               xs[:], start=True, stop=True)
            nc.scalar.activation(a1v[:, :, 1 + p_, 1 + q_], ps[:], Relu,
                                 bias=b1s[:])

    Add = mybir.AluOpType.add
    Max = mybir.AluOpType.max

    def relu_bias(engine_is_act, dest, src, bias):
        """dest = relu(src + bias), bias per-partition."""
        if engine_is_act:
            nc.scalar.activation(dest, src, Relu, bias=bias)
        else:
            nc.vector.tensor_scalar(dest, src, bias, 0.0, Add, Max)

    # ---- L2: a1 [16, t, 3, 3] -> a2rep [4 shift groups x 32ch, t, 9, 9].
    # The matmul M dim covers all 4 groups at once (w2 tap weights tiled
    # 4x), and 4 per-group scatters write the (di,dj)-shifted padded
    # layouts group g = di*2+dj needs so one K=128 matmul per L3 class
    # contracts every tap (w3c zeroes the groups a class doesn't use).
    a2 = a2p.tile([128, T * 81], f32, tag="a2")
    nc.gpsimd.memset(a2[:], 0.0)
    a2v = a2[:].rearrange("p (t i j) -> p t i j", t=T, i=9, j=9)
    for ry in (0, 1):
        nu = 4 if ry == 0 else 3
        for rx in (0, 1):
            nv = 4 if rx == 0 else 3
            taps = [(ky, kx) for ky in _class_taps(ry) for kx in _class_taps(rx)]
            for t0 in range(0, T, TG2):
                ps = p2p.tile([128, TG2 * nu * nv], f32, tag="p2")
                for i, (ky, kx) in enumerate(taps):
                    pp0 = 1 if ky == ry else 0
                    qq0 = 1 if kx == rx else 0
                    rhs = a1v[:, t0:t0 + TG2, pp0:pp0 + nu, qq0:qq0 + nv]
                    tap = ky * 3 + kx
                    mm(ps[:], w2s[:][:, tap * 128:(tap + 1) * 128],
                       rhs, start=(i == 0), stop=(i == len(taps) - 1))
                psv = ps[:].rearrange("p (t u v) -> p t u v",
                                      t=TG2, u=nu, v=nv)
                for g in range(4):
                    di, dj = g >> 1, g & 1
                    dest = a2v[32 * g:32 * (g + 1), t0:t0 + TG2,
                               1 + ry - di:1 + ry - di + 2 * nu:2,
                               1 + rx - dj:1 + rx - dj + 2 * nv:2]
                    relu_bias(g % 2 == 0, dest,
                              psv[32 * g:32 * (g + 1)],
                              b2s[:][32 * g:32 * (g + 1)])

    # ---- L3: a2rep -> out [64, t, 16, 16], chunked by CHUNK tokens;
    # each chunk DMA'd to its output row-group destination(s).
    for c0 in range(0, T, CHUNK):
        oc = ocp.tile([64, CHUNK * 256], f32, tag="oc")
        ocv = oc[:].rearrange("p (t y x) -> p t y x", t=CHUNK, y=16, x=16)
        for ry in (0, 1):
            for rx in (0, 1):
                cls = ry * 2 + rx
                for t0 in range(c0, c0 + CHUNK, TG3):
                    ps = p3p.tile([64, TG3 * 64], f32, tag="p3")
                    rhs = a2v[:, t0:t0 + TG3, 0:8, 0:8]
                    mm(ps[:], w3s[:][:, cls * 64:(cls + 1) * 64],
                       rhs, start=True, stop=True)
                    psv = ps[:].rearrange("p (t u v) -> p t u v",
                                          t=TG3, u=8, v=8)
                    dest = ocv[:, t0 - c0:t0 - c0 + TG3, ry::2, rx::2]
                    relu_bias(cls in (0, 3), dest, psv, b3s[:])
        ocs = oc[:].rearrange("p (t yx) -> p t yx", t=CHUNK, yx=256)
        reps = (0,) if phase == 0 else tuple(range(1, REPS))
        for rep in reps:
            row0 = rep * T + c0
            nc.sync.dma_start(out_v[:, row0:row0 + CHUNK, :], ocs)


def _build(repeats=1, internal_out=False, loop_r=0):
    """internal_out=True: write to an internal DRAM scratch tensor and
    expose only a tiny dummy ExternalOutput -- same instruction stream and
    HBM traffic, but avoids shuffling 512MB through PJRT per call (used
    for low-noise wall-clock timing)."""
    from contextlib import ExitStack

    import concourse.bacc as bacc
    import concourse.tile as tile
    from concourse import mybir

    f32 = mybir.dt.float32
    nc = bacc.Bacc("TRN2", target_bir_lowering=False, debug=False)
    xa = nc.dram_tensor("xa", [3, T], f32, kind="ExternalInput").ap()
    xb = nc.dram_tensor("xb", [3, T], f32, kind="ExternalInput").ap()
    w1d = nc.dram_tensor("w1r", [3, 144], f32, kind="ExternalInput").ap()
    w2d = nc.dram_tensor("w2r", [16, 9 * 128], f32, kind="ExternalInput").ap()
    w3d = nc.dram_tensor("w3r", [128, 256], f32, kind="ExternalInput").ap()
    b1d = nc.dram_tensor("b1c", [16, 1], f32, kind="ExternalInput").ap()
    b2d = nc.dram_tensor("b2c", [128, 1], f32, kind="ExternalInput").ap()
    b3d = nc.dram_tensor("b3c", [64, 1], f32, kind="ExternalInput").ap()
    if internal_out:
        out = nc.dram_tensor("outbuf", [REPS * T, 64 * 256], f32).ap()
        dummy = nc.dram_tensor("tinyout", [1, 4], f32,
                               kind="ExternalOutput").ap()
    else:
        out = nc.dram_tensor("out", [REPS * T, 64 * 256], f32,
                             kind="ExternalOutput").ap()
        dummy = None
    out_v = out.rearrange("r (o yx) -> o r yx", o=64, yx=256)

    with tile.TileContext(nc) as tc:
        with ExitStack() as ctx:
            const = ctx.enter_context(tc.tile_pool(name="const", bufs=1))
            a1p = ctx.enter_context(tc.tile_pool(name="a1", bufs=2))
            a2p = ctx.enter_context(tc.tile_pool(name="a2", bufs=2))
            ocp = ctx.enter_context(tc.tile_pool(name="oc", bufs=3))
            p1p = ctx.enter_context(tc.tile_pool(name="p1", bufs=2,
                                                 space="PSUM"))
            p2p = ctx.enter_context(tc.tile_pool(name="p2", bufs=2,
                                                 space="PSUM"))
            p3p = ctx.enter_context(tc.tile_pool(name="p3", bufs=4,
                                                 space="PSUM"))
            pools = (a1p, a2p, ocp, p1p, p2p, p3p)

            def load_const(shape, src, tag):
                t = const.tile(shape, f32, tag=tag)
                nc.sync.dma_start(t[:], src)
                return t

            w1s = load_const([3, 144], w1d, "w1")
            w2s = load_const([16, 9 * 128], w2d, "w2")
            w3s = load_const([128, 256], w3d, "w3")
            b1s = load_const([16, 1], b1d, "b1")
            b2s = load_const([128, 1], b2d, "b2")
            b3s = load_const([64, 1], b3d, "b3")
            xas = load_const([3, T], xa, "xa")
            xbs = load_const([3, T], xb, "xb")

            def emit_body():
                for _rep in range(repeats):
                    for phase, xs in ((0, xas), (1, xbs)):
                        _build_phase(nc, tc, ctx, mybir, xs, w1s, w2s, w3s,
                                     b1s, b2s, b3s, pools, out_v, phase)

            if loop_r:
                with tc.For_i(0, loop_r, 1):
                    emit_body()
            else:
                emit_body()
            if dummy is not None:
                nc.sync.dma_start(dummy, w1s[:][0:1, 0:4])
    nc.compile()
    return nc


def _get_nc():
    if "nc" not in _cache:
        _cache["nc"] = _build()
    return _cache["nc"]


def _stack_w3(w3):
    """Build the K-stacked L3 weight matrix [128, 256]: row (g, c) with
    shift group g = di*2+dj, column (cls, o) with class cls = ry*2+rx.
    Entry = w3[c, o, ky, kx] for the class tap whose padded-input shift
    is (di, dj); groups a class doesn't use stay zero."""
    w3c = np.zeros((128, 256), np.float32)
    for ry in (0, 1):
        for rx in (0, 1):
            cls = ry * 2 + rx
            for ky in _class_taps(ry):
                for kx in _class_taps(rx):
                    di = 1 if ky == ry else 0
                    dj = 1 if kx == rx else 0
                    g = di * 2 + dj
                    w3c[32 * g:32 * (g + 1), 64 * cls:64 * (cls + 1)] = \
                        w3[:, :, ky, kx]
    return w3c


def _prep_in_maps(hist_traj, w1, b1, w2, b2, w3, b3):
    h = np.where(hist_traj == -1.0, 0.0, hist_traj).astype(np.float32)
    xA = np.ascontiguousarray(h[0].T)  # [3, T]
    xB = np.ascontiguousarray(h[1].T)
    common = {
        "w1r": np.ascontiguousarray(
            w1.transpose(0, 2, 3, 1).reshape(3, 144)).astype(np.float32),
        "w2r": np.ascontiguousarray(np.tile(
            w2.transpose(0, 2, 3, 1).reshape(16, 9, 1, 32),
            (1, 1, 4, 1)).reshape(16, 9 * 128)).astype(np.float32),
        "w3r": _stack_w3(w3),
        "b1c": np.ascontiguousarray(b1.reshape(16, 1)).astype(np.float32),
        "b2c": np.ascontiguousarray(np.tile(b2.reshape(1, 32),
            (4, 1)).reshape(128, 1)).astype(np.float32),
        "b3c": np.ascontiguousarray(b3.reshape(64, 1)).astype(np.float32),
        "xb": xB,
    }
    in_maps = []
    for core in range(N_CORES):
        m = dict(common)
        m["xa"] = xA if core == 0 else xB
        in_maps.append(m)
    return in_maps


def run_on_hw(hist_traj, w1, b1, w2, b2, w3, b3, trace=False,
              trace_cores=None):
    """Returns ((enc_out, seq_len), BassKernelResults)."""
    from concourse.bass_utils import run_bass_kernel_spmd

    nc = _get_nc()
    in_maps = _prep_in_maps(hist_traj, w1, b1, w2, b2, w3, b3)
    res = run_bass_kernel_spmd(nc, in_maps, list(range(N_CORES)),
                               trace=trace, trace_cores=trace_cores)
    enc = np.concatenate([r["out"] for r in res.results], axis=0)
    enc = enc.reshape(B * T, 64, 16, 16)
    seq_len = np.full((B,), T, dtype=np.int32)
    return (enc, seq_len), res


def kernel(hist_traj, w1, b1, w2, b2, w3, b3):
    out, _ = run_on_hw(hist_traj, w1, b1, w2, b2, w3, b3)
    return out


# revision 20
# speedup vs baseline: 1.6891x; 1.1596x over previous
"""Trainium2 Bass kernel for nn_Encoder_74947179316057.

Reference computation: pack hist_traj [64,128,3] to 8192 tokens, run each
token vector through three stride-2 ConvTranspose2d+ReLU layers
(1x1 -> 3x3 -> 7x7 -> 16x16), then the reference's un-slicing makes the
output depend only on the first 2*T tokens:
  enc_out[0:T]   = f(tokens of batch 0)           (block A)
  enc_out[T:B*T] = f(tokens of batch 1) tiled 63x (block B)

Kernel strategy (8 cores, data-parallel over output rows):
  core k owns output rows [k*1024, (k+1)*1024) = 8 row-groups of T=128.
  Every core computes f() for two 128-token blocks (xa, xb) fully on-chip
  and writes its 64MB output shard; block-B results are written to 7
  row-group destinations straight out of SBUF, so HBM traffic is just the
  output (memory-bound regime).

Each ConvTranspose2d(stride 2, k=3) is decomposed by output-pixel parity
class (y%2, x%2): class (ry,rx) output [Cout, t, nu, nv] is a PSUM-accumulated
sum over kernel taps (ky,kx) with ky=ry or ry+2, kx=rx or rx+2, each tap a
matmul of w[:, :, ky, kx] (lhsT [Cin,Cout]) against a shifted window of the
zero-padded previous activation (rhs [Cin, t*nu*nv]).  ReLU+bias happen in
the PSUM->SBUF scatter (scalar engine activation with per-partition bias).
"""

import os
import sys

import numpy as np

for _p in ("/opt/trn_rl_repo", "/root/.axon_site/_ro/trn_rl_repo"):
    if os.path.isdir(_p) and _p not in sys.path:
        sys.path.append(_p)

B, T, C = 64, 128, 3
N_CORES = 8
REPS = (B * T) // N_CORES // T  # 8 row-groups of T rows per core
CHUNK = 16   # tokens per L3 output chunk (SBUF staging tile)
TG2 = 32     # tokens per L2 psum group (32*16 pix = 512 max free)
TG3 = 8      # tokens per L3 psum group (8*64 pix = 512 free)

_cache = {}

# Matmul input dtype: float32r streams fp32 data faster through the PE
# but is reduced precision and needs an explicit rounding producer
# (walrus BIR verifier rejects plain-fp32-fed fp32r matmuls).  Keep
# exact fp32; PE cost is managed by K-stacking taps instead.
MM_F32R = False


def _class_taps(r):
    """Kernel taps contributing to output parity class r (stride 2, k=3)."""
    return (r, r + 2) if r == 0 else (r,)


def _build_phase(nc, tc, ctx, mybir, xs, w1s, w2s, w3s, b1s, b2s, b3s,
                 pools, out_v, phase):
    f32 = mybir.dt.float32
    Relu = mybir.ActivationFunctionType.Relu
    a1p, a2p, ocp, p1p, p2p, p3p = pools

    def mm(out, lhsT, rhs, start, stop):
        if MM_F32R:
            lhsT = lhsT.bitcast(mybir.dt.float32r)
            rhs = rhs.bitcast(mybir.dt.float32r)
        nc.tensor.matmul(out, lhsT, rhs, start=start, stop=stop)

    Add = mybir.AluOpType.add
    Max = mybir.AluOpType.max

    def relu_bias(engine_is_act, dest, src, bias):
        """dest = relu(src + bias), bias per-partition."""
        if engine_is_act:
            nc.scalar.activation(dest, src, Relu, bias=bias)
        else:
            nc.vector.tensor_scalar(dest, src, bias, 0.0, Add, Max)

    # ---- L1: token vec [3] -> a1rep [4 shift groups x 16ch, t, 5, 5]
    # (zero-padded 5x5, group g = di*2+dj holds a1pad[c,t,i+di,j+dj]).
    a1 = a1p.tile([128, T * 25], f32, tag="a1")
    nc.vector.memset(a1[:], 0.0)
    a1v = a1[:].rearrange("p (t i j) -> p t i j", t=T, i=5, j=5)
    for p_ in range(3):
        for q_ in range(3):
            pos = p_ * 3 + q_
            ps = p1p.tile([128, T], f32, tag="p1")
            mm(ps[:], w1s[:][:, pos * 128:(pos + 1) * 128],
               xs[:], start=True, stop=True)
            for g in range(4):
                di, dj = g >> 1, g & 1
                dest = a1v[32 * g:32 * g + 16, :, 1 + p_ - di, 1 + q_ - dj]
                relu_bias(g % 2 == 0, dest,
                          ps[:][32 * g:32 * g + 16],
                          b1s[:][32 * g:32 * g + 16])

    # ---- L2: a1rep -> a2rep [4 shift groups x 32ch, t, 9, 9].  One
    # K=64 matmul per (class, token group) contracts all taps at once
    # (w2c zeroes the a1 groups a class doesn't use); matmul M covers
    # all 4 a2 output groups, and 4 per-group scatters write the
    # (di,dj)-shifted padded layouts so L3 can do the same K=128 trick.
    a2 = a2p.tile([128, T * 81], f32, tag="a2")
    nc.gpsimd.memset(a2[:], 0.0)
    a2v = a2[:].rearrange("p (t i j) -> p t i j", t=T, i=9, j=9)
    for ry in (0, 1):
        nu = 4 if ry == 0 else 3
        for rx in (0, 1):
            nv = 4 if rx == 0 else 3
            cls = ry * 2 + rx
            for t0 in range(0, T, TG2):
                ps = p2p.tile([128, TG2 * nu * nv], f32, tag="p2")
                rhs = a1v[:, t0:t0 + TG2, 0:nu, 0:nv]
                mm(ps[:], w2s[:][:, cls * 128:(cls + 1) * 128],
                   rhs, start=True, stop=True)
                psv = ps[:].rearrange("p (t u v) -> p t u v",
                                      t=TG2, u=nu, v=nv)
                for g in range(4):
                    di, dj = g >> 1, g & 1
                    dest = a2v[32 * g:32 * (g + 1), t0:t0 + TG2,
                               1 + ry - di:1 + ry - di + 2 * nu:2,
                               1 + rx - dj:1 + rx - dj + 2 * nv:2]
                    relu_bias(g % 2 == 0, dest,
                              psv[32 * g:32 * (g + 1)],
                              b2s[:][32 * g:32 * (g + 1)])

    # ---- L3: a2rep -> out [64, t, 16, 16], chunked by CHUNK tokens;
    # each chunk DMA'd to its output row-group destination(s).
    for c0 in range(0, T, CHUNK):
        oc = ocp.tile([64, CHUNK * 256], f32, tag="oc")
        ocv = oc[:].rearrange("p (t y x) -> p t y x", t=CHUNK, y=16, x=16)
        for ry in (0, 1):
            for rx in (0, 1):
                cls = ry * 2 + rx
                for t0 in range(c0, c0 + CHUNK, TG3):
                    ps = p3p.tile([64, TG3 * 64], f32, tag="p3")
                    rhs = a2v[:, t0:t0 + TG3, 0:8, 0:8]
                    mm(ps[:], w3s[:][:, cls * 64:(cls + 1) * 64],
                       rhs, start=True, stop=True)
                    psv = ps[:].rearrange("p (t u v) -> p t u v",
                                          t=TG3, u=8, v=8)
                    dest = ocv[:, t0 - c0:t0 - c0 + TG3, ry::2, rx::2]
                    relu_bias(cls in (0, 3), dest, psv, b3s[:])
        ocs = oc[:].rearrange("p (t yx) -> p t yx", t=CHUNK, yx=256)
        reps = (0,) if phase == 0 else tuple(range(1, REPS))
        for rep in reps:
            row0 = rep * T + c0
            nc.sync.dma_start(out_v[:, row0:row0 + CHUNK, :], ocs)


def _build(repeats=1, internal_out=False, loop_r=0):
    """internal_out=True: write to an internal DRAM scratch tensor and
    expose only a tiny dummy ExternalOutput -- same instruction stream and
    HBM traffic, but avoids shuffling 512MB through PJRT per call (used
    for low-noise wall-clock timing)."""
    from contextlib import ExitStack

    import concourse.bacc as bacc
    import concourse.tile as tile
    from concourse import mybir

    f32 = mybir.dt.float32
    nc = bacc.Bacc("TRN2", target_bir_lowering=False, debug=False)
    xa = nc.dram_tensor("xa", [3, T], f32, kind="ExternalInput").ap()
    xb = nc.dram_tensor("xb", [3, T], f32, kind="ExternalInput").ap()
    w1d = nc.dram_tensor("w1r", [3, 9 * 128], f32, kind="ExternalInput").ap()
    w2d = nc.dram_tensor("w2r", [128, 4 * 128], f32, kind="ExternalInput").ap()
    w3d = nc.dram_tensor("w3r", [128, 256], f32, kind="ExternalInput").ap()
    b1d = nc.dram_tensor("b1c", [128, 1], f32, kind="ExternalInput").ap()
    b2d = nc.dram_tensor("b2c", [128, 1], f32, kind="ExternalInput").ap()
    b3d = nc.dram_tensor("b3c", [64, 1], f32, kind="ExternalInput").ap()
    if internal_out:
        out = nc.dram_tensor("outbuf", [REPS * T, 64 * 256], f32).ap()
        dummy = nc.dram_tensor("tinyout", [1, 4], f32,
                               kind="ExternalOutput").ap()
    else:
        out = nc.dram_tensor("out", [REPS * T, 64 * 256], f32,
                             kind="ExternalOutput").ap()
        dummy = None
    out_v = out.rearrange("r (o yx) -> o r yx", o=64, yx=256)

    with tile.TileContext(nc) as tc:
        with ExitStack() as ctx:
            const = ctx.enter_context(tc.tile_pool(name="const", bufs=1))
            a1p = ctx.enter_context(tc.tile_pool(name="a1", bufs=2))
            a2p = ctx.enter_context(tc.tile_pool(name="a2", bufs=2))
            ocp = ctx.enter_context(tc.tile_pool(name="oc", bufs=3))
            p1p = ctx.enter_context(tc.tile_pool(name="p1", bufs=2,
                                                 space="PSUM"))
            p2p = ctx.enter_context(tc.tile_pool(name="p2", bufs=2,
                                                 space="PSUM"))
            p3p = ctx.enter_context(tc.tile_pool(name="p3", bufs=4,
                                                 space="PSUM"))
            pools = (a1p, a2p, ocp, p1p, p2p, p3p)

            def load_const(shape, src, tag):
                t = const.tile(shape, f32, tag=tag)
                nc.sync.dma_start(t[:], src)
                return t

            w1s = load_const([3, 9 * 128], w1d, "w1")
            w2s = load_const([128, 4 * 128], w2d, "w2")
            w3s = load_const([128, 256], w3d, "w3")
            b1s = load_const([128, 1], b1d, "b1")
            b2s = load_const([128, 1], b2d, "b2")
            b3s = load_const([64, 1], b3d, "b3")
            xas = load_const([3, T], xa, "xa")
            xbs = load_const([3, T], xb, "xb")

            def emit_body():
                for _rep in range(repeats):
                    for phase, xs in ((0, xas), (1, xbs)):
                        _build_phase(nc, tc, ctx, mybir, xs, w1s, w2s, w3s,
                                     b1s, b2s, b3s, pools, out_v, phase)

            if loop_r:
                with tc.For_i(0, loop_r, 1):
                    emit_body()
            else:
                emit_body()
            if dummy is not None:
                nc.sync.dma_start(dummy, w1s[:][0:1, 0:4])
    nc.compile()
    return nc


def _get_nc():
    if "nc" not in _cache:
        _cache["nc"] = _build()
    return _cache["nc"]


def _pad_groups(vec, used):
    """Tile a [used]-vector into 4 x 32-partition groups (rest zero)."""
    out = np.zeros((4, 32), np.float32)
    out[:, :used] = np.asarray(vec, np.float32).reshape(1, used)
    return out.reshape(128)


def _stack_w1(w1):
    """L1 weights [3, 9*128]: per position pos, M-block (g*32 + o) with
    o < 16 = w1[:, o, p, q] replicated over the 4 shift groups g."""
    w1c = np.zeros((3, 9, 128), np.float32)
    for p in range(3):
        for q in range(3):
            for g in range(4):
                w1c[:, p * 3 + q, 32 * g:32 * g + 16] = w1[:, :, p, q]
    return np.ascontiguousarray(w1c.reshape(3, 9 * 128))


def _stack_w2(w2):
    """K-stacked L2 weights [128, 512]: row (g1*32 + c1) with input
    shift group g1 (c1 < 16), column (cls, g2, c2).  Entry =
    w2[c1, c2, ky, kx] for the class tap whose shift is g1 (all output
    groups g2 get the same)."""
    w2c = np.zeros((128, 512), np.float32)
    for ry in (0, 1):
        for rx in (0, 1):
            cls = ry * 2 + rx
            for ky in _class_taps(ry):
                for kx in _class_taps(rx):
                    di = 1 if ky == ry else 0
                    dj = 1 if kx == rx else 0
                    g1 = di * 2 + dj
                    blk = w2[:, :, ky, kx]          # [16, 32]
                    for g2 in range(4):
                        w2c[32 * g1:32 * g1 + 16,
                            cls * 128 + 32 * g2:cls * 128 + 32 * (g2 + 1)] = blk
    return w2c


def _stack_w3(w3):
    """Build the K-stacked L3 weight matrix [128, 256]: row (g, c) with
    shift group g = di*2+dj, column (cls, o) with class cls = ry*2+rx.
    Entry = w3[c, o, ky, kx] for the class tap whose padded-input shift
    is (di, dj); groups a class doesn't use stay zero."""
    w3c = np.zeros((128, 256), np.float32)
    for ry in (0, 1):
        for rx in (0, 1):
            cls = ry * 2 + rx
            for ky in _class_taps(ry):
                for kx in _class_taps(rx):
                    di = 1 if ky == ry else 0
                    dj = 1 if kx == rx else 0
                    g = di * 2 + dj
                    w3c[32 * g:32 * (g + 1), 64 * cls:64 * (cls + 1)] = \
                        w3[:, :, ky, kx]
    return w3c


def _prep_in_maps(hist_traj, w1, b1, w2, b2, w3, b3):
    h = np.where(hist_traj == -1.0, 0.0, hist_traj).astype(np.float32)
    xA = np.ascontiguousarray(h[0].T)  # [3, T]
    xB = np.ascontiguousarray(h[1].T)
    common = {
        "w1r": _stack_w1(w1),
        "w2r": _stack_w2(w2),
        "w3r": _stack_w3(w3),
        "b1c": _pad_groups(b1, 16).reshape(128, 1),
        "b2c": np.ascontiguousarray(np.tile(b2.reshape(1, 32),
            (4, 1)).reshape(128, 1)).astype(np.float32),
        "b3c": np.ascontiguousarray(b3.reshape(64, 1)).astype(np.float32),
        "xb": xB,
    }
    in_maps = []
    for core in range(N_CORES):
        m = dict(common)
        m["xa"] = xA if core == 0 else xB
        in_maps.append(m)
    return in_maps


def run_on_hw(hist_traj, w1, b1, w2, b2, w3, b3, trace=False,
              trace_cores=None):
    """Returns ((enc_out, seq_len), BassKernelResults)."""
    from concourse.bass_utils import run_bass_kernel_spmd

    nc = _get_nc()
    in_maps = _prep_in_maps(hist_traj, w1, b1, w2, b2, w3, b3)
    res = run_bass_kernel_spmd(nc, in_maps, list(range(N_CORES)),
                               trace=trace, trace_cores=trace_cores)
    enc = np.concatenate([r["out"] for r in res.results], axis=0)
    enc = enc.reshape(B * T, 64, 16, 16)
    seq_len = np.full((B,), T, dtype=np.int32)
    return (enc, seq_len), res


def kernel(hist_traj, w1, b1, w2, b2, w3, b3):
    out, _ = run_on_hw(hist_traj, w1, b1, w2, b2, w3, b3)
    return out


# revision 27
# speedup vs baseline: 2.0084x; 1.1890x over previous
"""Trainium2 Bass kernel for nn_Encoder_74947179316057.

Reference computation: pack hist_traj [64,128,3] to 8192 tokens, run each
token vector through three stride-2 ConvTranspose2d+ReLU layers
(1x1 -> 3x3 -> 7x7 -> 16x16), then the reference's un-slicing makes the
output depend only on the first 2*T tokens:
  enc_out[0:T]   = f(tokens of batch 0)           (block A)
  enc_out[T:B*T] = f(tokens of batch 1) tiled 63x (block B)

Kernel strategy (8 cores, data-parallel over output rows):
  core k owns output rows [k*1024, (k+1)*1024) = 8 row-groups of T=128.
  Every core computes f() for two 128-token blocks (xa, xb) fully on-chip
  and writes its 64MB output shard; block-B results are written to 7
  row-group destinations straight out of SBUF, so HBM traffic is just the
  output (memory-bound regime).

Each ConvTranspose2d(stride 2, k=3) is decomposed by output-pixel parity
class (y%2, x%2): class (ry,rx) output [Cout, t, nu, nv] is a PSUM-accumulated
sum over kernel taps (ky,kx) with ky=ry or ry+2, kx=rx or rx+2, each tap a
matmul of w[:, :, ky, kx] (lhsT [Cin,Cout]) against a shifted window of the
zero-padded previous activation (rhs [Cin, t*nu*nv]).  ReLU+bias happen in
the PSUM->SBUF scatter (scalar engine activation with per-partition bias).
"""

import os
import sys

import numpy as np

for _p in ("/opt/trn_rl_repo", "/root/.axon_site/_ro/trn_rl_repo"):
    if os.path.isdir(_p) and _p not in sys.path:
        sys.path.append(_p)

B, T, C = 64, 128, 3
N_CORES = 8
REPS = (B * T) // N_CORES // T  # 8 row-groups of T rows per core
CHUNK = 16   # tokens per L3 output chunk (SBUF staging tile)
TG2 = 32     # tokens per L2 psum group (32*16 pix = 512 max free)
TG3 = 8      # tokens per L3 psum group (8*64 pix = 512 free)

_cache = {}

# Matmul input dtype: float32r streams fp32 data faster through the PE
# but is reduced precision and needs an explicit rounding producer
# (walrus BIR verifier rejects plain-fp32-fed fp32r matmuls).  Keep
# exact fp32; PE cost is managed by K-stacking taps instead.
MM_F32R = False

# Token-major L3: stage the output [token, o*y*x] so DRAM writes are
# 64KB-contiguous per token row (~328 GB/s vs ~195 GB/s for the o-major
# 1KB-run pattern the feature-major L3 produces).
TM = True


def _class_taps(r):
    """Kernel taps contributing to output parity class r (stride 2, k=3)."""
    return (r, r + 2) if r == 0 else (r,)


def _build_phase(nc, tc, ctx, mybir, xs, w1s, w2s, w3s, b1s, b2s, b3s,
                 pools, out_v, phase):
    f32 = mybir.dt.float32
    Relu = mybir.ActivationFunctionType.Relu
    a1p, a2p, ocp, p1p, p2p, p3p = pools

    def mm(out, lhsT, rhs, start, stop):
        if MM_F32R:
            lhsT = lhsT.bitcast(mybir.dt.float32r)
            rhs = rhs.bitcast(mybir.dt.float32r)
        nc.tensor.matmul(out, lhsT, rhs, start=start, stop=stop)

    Add = mybir.AluOpType.add
    Max = mybir.AluOpType.max

    def relu_bias(engine_is_act, dest, src, bias):
        """dest = relu(src + bias), bias per-partition."""
        if engine_is_act:
            nc.scalar.activation(dest, src, Relu, bias=bias)
        else:
            nc.vector.tensor_scalar(dest, src, bias, 0.0, Add, Max)

    # ---- L1: token vec [3] -> a1rep [4 shift groups x 16ch, t, 5, 5]
    # (zero-padded 5x5, group g = di*2+dj holds a1pad[c,t,i+di,j+dj]).
    a1 = a1p.tile([128, T * 25], f32, tag="a1")
    nc.vector.memset(a1[:], 0.0)
    a1v = a1[:].rearrange("p (t i j) -> p t i j", t=T, i=5, j=5)
    for p_ in range(3):
        for q_ in range(3):
            pos = p_ * 3 + q_
            ps = p1p.tile([128, T], f32, tag="p1")
            mm(ps[:], w1s[:][:, pos * 128:(pos + 1) * 128],
               xs[:], start=True, stop=True)
            for g in range(4):
                di, dj = g >> 1, g & 1
                dest = a1v[32 * g:32 * g + 16, :, 1 + p_ - di, 1 + q_ - dj]
                relu_bias(g % 2 == 0, dest,
                          ps[:][32 * g:32 * g + 16],
                          b1s[:][32 * g:32 * g + 16])

    # ---- L2: a1rep -> a2rep [4 shift groups x 32ch, t, 9, 9].  One
    # K=64 matmul per (class, token group) contracts all taps at once
    # (w2c zeroes the a1 groups a class doesn't use); matmul M covers
    # all 4 a2 output groups, and 4 per-group scatters write the
    # (di,dj)-shifted padded layouts so L3 can do the same K=128 trick.
    # TM: a2rep groups are j-shifts g'=0..3 (packed j' 0..5), so L3 can
    # read [K=(g',c), t] activation columns token-major.  FM: groups are
    # the 4 (di,dj) tap shifts for the K-stacked feature-major L3.
    JW = 6 if TM else 9
    a2 = a2p.tile([128, T * 9 * JW], f32, tag="a2")
    nc.gpsimd.memset(a2[:], 0.0)
    a2v = a2[:].rearrange("p (t i j) -> p t i j", t=T, i=9, j=JW)
    for ry in (0, 1):
        nu = 4 if ry == 0 else 3
        for rx in (0, 1):
            nv = 4 if rx == 0 else 3
            cls = ry * 2 + rx
            for t0 in range(0, T, TG2):
                ps = p2p.tile([128, TG2 * nu * nv], f32, tag="p2")
                rhs = a1v[:, t0:t0 + TG2, 0:nu, 0:nv]
                mm(ps[:], w2s[:][:, cls * 128:(cls + 1) * 128],
                   rhs, start=True, stop=True)
                psv = ps[:].rearrange("p (t u v) -> p t u v",
                                      t=TG2, u=nu, v=nv)
                for g in range(4):
                    if TM:
                        v_min = max(0, (g - rx) // 2)
                        v_max = min(nv - 1, (4 + g - rx) // 2)
                        if v_min > v_max:
                            continue
                        j0 = 1 + rx + 2 * v_min - g
                        nvg = v_max - v_min + 1
                        dest = a2v[32 * g:32 * (g + 1), t0:t0 + TG2,
                                   1 + ry:1 + ry + 2 * nu:2,
                                   j0:j0 + 2 * nvg - 1:2]
                        src = psv[32 * g:32 * (g + 1), :, :,
                                  v_min:v_max + 1]
                    else:
                        di, dj = g >> 1, g & 1
                        dest = a2v[32 * g:32 * (g + 1), t0:t0 + TG2,
                                   1 + ry - di:1 + ry - di + 2 * nu:2,
                                   1 + rx - dj:1 + rx - dj + 2 * nv:2]
                        src = psv[32 * g:32 * (g + 1)]
                    relu_bias(g % 2 == 0, dest, src,
                              b2s[:][32 * g:32 * (g + 1)])

    # ---- L3 (token-major): lhsT = a2rep column slices [K=128, M=t],
    # rhs = per-tap block-diagonal weights [128, (vpos, o)]; psum
    # [t, 256] scattered (+bias) into obuf [t, o*y*x] so the output DMA
    # writes 64KB-contiguous DRAM rows (1KB-run o-major writes only
    # reach ~195 GB/s vs ~328 GB/s contiguous).
    if TM:
        obuf = ocp.tile([128, 16384], f32, tag="ob")
        obv = obuf[:].rearrange("p (o y x) -> p o y x", o=64, y=16, x=16)
        b3v = b3s[:].rearrange("p (g o) -> p g o", g=4, o=64)
        for ry in (0, 1):
            for rx in (0, 1):
                taps = [(ky, kx) for ky in _class_taps(ry)
                        for kx in _class_taps(rx)]
                for u in range(8):
                    for v0 in (0, 4):
                        ps = p3p.tile([128, 256], f32, tag="p3")
                        for i, (ky, kx) in enumerate(taps):
                            pp0 = 1 if ky == ry else 0
                            qq0 = 1 if kx == rx else 0
                            tap = ky * 3 + kx
                            lhsT = a2v[:, :, u + pp0, v0 + qq0]
                            mm(ps[:], lhsT,
                               w3s[:][:, tap * 256:(tap + 1) * 256],
                               start=(i == 0), stop=(i == len(taps) - 1))
                        psv = ps[:].rearrange("p (g o) -> p g o", g=4, o=64)
                        dst = obv[:, :, 2 * u + ry,
                                  2 * v0 + rx:2 * v0 + rx + 7:2]
                        nc.vector.tensor_tensor(
                            dst.transpose([0, 2, 1]), psv, b3v,
                            op=mybir.AluOpType.add)
        nc.scalar.activation(obuf[:], obuf[:], Relu)
        reps = (0,) if phase == 0 else tuple(range(1, REPS))
        for rep in reps:
            nc.sync.dma_start(out_v[rep * T:rep * T + T, :], obuf[:])
        return

    # ---- L3 (feature-major): a2rep -> out [64, t, 16, 16], chunked by
    # CHUNK tokens; each chunk DMA'd to its row-group destination(s).
    for c0 in range(0, T, CHUNK):
        oc = ocp.tile([64, CHUNK * 256], f32, tag="oc")
        ocv = oc[:].rearrange("p (t y x) -> p t y x", t=CHUNK, y=16, x=16)
        for ry in (0, 1):
            for rx in (0, 1):
                cls = ry * 2 + rx
                for t0 in range(c0, c0 + CHUNK, TG3):
                    ps = p3p.tile([64, TG3 * 64], f32, tag="p3")
                    rhs = a2v[:, t0:t0 + TG3, 0:8, 0:8]
                    mm(ps[:], w3s[:][:, cls * 64:(cls + 1) * 64],
                       rhs, start=True, stop=True)
                    psv = ps[:].rearrange("p (t u v) -> p t u v",
                                          t=TG3, u=8, v=8)
                    dest = ocv[:, t0 - c0:t0 - c0 + TG3, ry::2, rx::2]
                    relu_bias(cls in (0, 3), dest, psv, b3s[:])
        ocs = oc[:].rearrange("p (t yx) -> p t yx", t=CHUNK, yx=256)
        reps = (0,) if phase == 0 else tuple(range(1, REPS))
        for rep in reps:
            row0 = rep * T + c0
            nc.sync.dma_start(out_v[:, row0:row0 + CHUNK, :], ocs)


def _build(repeats=1, internal_out=False, loop_r=0):
    """internal_out=True: write to an internal DRAM scratch tensor and
    expose only a tiny dummy ExternalOutput -- same instruction stream and
    HBM traffic, but avoids shuffling 512MB through PJRT per call (used
    for low-noise wall-clock timing)."""
    from contextlib import ExitStack

    import concourse.bacc as bacc
    import concourse.tile as tile
    from concourse import mybir

    f32 = mybir.dt.float32
    nc = bacc.Bacc("TRN2", target_bir_lowering=False, debug=False)
    xa = nc.dram_tensor("xa", [3, T], f32, kind="ExternalInput").ap()
    xb = nc.dram_tensor("xb", [3, T], f32, kind="ExternalInput").ap()
    w1d = nc.dram_tensor("w1r", [3, 9 * 128], f32, kind="ExternalInput").ap()
    w2d = nc.dram_tensor("w2r", [128, 4 * 128], f32, kind="ExternalInput").ap()
    w3shape = [128, 9 * 256] if TM else [128, 256]
    b3shape = [128, 256] if TM else [64, 1]
    w3d = nc.dram_tensor("w3r", w3shape, f32, kind="ExternalInput").ap()
    b1d = nc.dram_tensor("b1c", [128, 1], f32, kind="ExternalInput").ap()
    b2d = nc.dram_tensor("b2c", [128, 1], f32, kind="ExternalInput").ap()
    b3d = nc.dram_tensor("b3c", b3shape, f32, kind="ExternalInput").ap()
    if internal_out:
        out = nc.dram_tensor("outbuf", [REPS * T, 64 * 256], f32).ap()
        dummy = nc.dram_tensor("tinyout", [1, 4], f32,
                               kind="ExternalOutput").ap()
    else:
        out = nc.dram_tensor("out", [REPS * T, 64 * 256], f32,
                             kind="ExternalOutput").ap()
        dummy = None
    # TM writes whole token rows (contiguous); FM scatters o-major.
    out_v = out if TM else out.rearrange("r (o yx) -> o r yx", o=64, yx=256)

    with tile.TileContext(nc) as tc:
        with ExitStack() as ctx:
            const = ctx.enter_context(tc.tile_pool(name="const", bufs=1))
            a1p = ctx.enter_context(tc.tile_pool(name="a1",
                                                 bufs=1 if TM else 2))
            a2p = ctx.enter_context(tc.tile_pool(name="a2",
                                                 bufs=1 if TM else 2))
            ocp = ctx.enter_context(tc.tile_pool(name="oc",
                                                 bufs=2 if TM else 3))
            p1p = ctx.enter_context(tc.tile_pool(name="p1", bufs=2,
                                                 space="PSUM"))
            p2p = ctx.enter_context(tc.tile_pool(name="p2", bufs=2,
                                                 space="PSUM"))
            p3p = ctx.enter_context(tc.tile_pool(name="p3", bufs=4,
                                                 space="PSUM"))
            pools = (a1p, a2p, ocp, p1p, p2p, p3p)

            def load_const(shape, src, tag):
                t = const.tile(shape, f32, tag=tag)
                nc.sync.dma_start(t[:], src)
                return t

            w1s = load_const([3, 9 * 128], w1d, "w1")
            w2s = load_const([128, 4 * 128], w2d, "w2")
            w3s = load_const(w3shape, w3d, "w3")
            b1s = load_const([128, 1], b1d, "b1")
            b2s = load_const([128, 1], b2d, "b2")
            b3s = load_const(b3shape, b3d, "b3")
            xas = load_const([3, T], xa, "xa")
            xbs = load_const([3, T], xb, "xb")

            def emit_body():
                # block B first: its 7 row-group writes are the bulk of
                # the HBM traffic, so start them as early as possible.
                for _rep in range(repeats):
                    for phase, xs in ((1, xbs), (0, xas)):
                        _build_phase(nc, tc, ctx, mybir, xs, w1s, w2s, w3s,
                                     b1s, b2s, b3s, pools, out_v, phase)

            if loop_r:
                with tc.For_i(0, loop_r, 1):
                    emit_body()
            else:
                emit_body()
            if dummy is not None:
                nc.sync.dma_start(dummy, w1s[:][0:1, 0:4])
    nc.compile()
    return nc


def _get_nc():
    if "nc" not in _cache:
        _cache["nc"] = _build()
    return _cache["nc"]


def _pad_groups(vec, used):
    """Tile a [used]-vector into 4 x 32-partition groups (rest zero)."""
    out = np.zeros((4, 32), np.float32)
    out[:, :used] = np.asarray(vec, np.float32).reshape(1, used)
    return out.reshape(128)


def _stack_w1(w1):
    """L1 weights [3, 9*128]: per position pos, M-block (g*32 + o) with
    o < 16 = w1[:, o, p, q] replicated over the 4 shift groups g."""
    w1c = np.zeros((3, 9, 128), np.float32)
    for p in range(3):
        for q in range(3):
            for g in range(4):
                w1c[:, p * 3 + q, 32 * g:32 * g + 16] = w1[:, :, p, q]
    return np.ascontiguousarray(w1c.reshape(3, 9 * 128))


def _stack_w2(w2):
    """K-stacked L2 weights [128, 512]: row (g1*32 + c1) with input
    shift group g1 (c1 < 16), column (cls, g2, c2).  Entry =
    w2[c1, c2, ky, kx] for the class tap whose shift is g1 (all output
    groups g2 get the same)."""
    w2c = np.zeros((128, 512), np.float32)
    for ry in (0, 1):
        for rx in (0, 1):
            cls = ry * 2 + rx
            for ky in _class_taps(ry):
                for kx in _class_taps(rx):
                    di = 1 if ky == ry else 0
                    dj = 1 if kx == rx else 0
                    g1 = di * 2 + dj
                    blk = w2[:, :, ky, kx]          # [16, 32]
                    for g2 in range(4):
                        w2c[32 * g1:32 * g1 + 16,
                            cls * 128 + 32 * g2:cls * 128 + 32 * (g2 + 1)] = blk
    return w2c


def _stack_w3(w3):
    """FM: K-stacked L3 weights [128, 256]: row (g, c) with shift group
    g = di*2+dj, column (cls, o); the class tap whose padded-input
    shift is (di, dj); groups a class doesn't use stay zero."""
    w3c = np.zeros((128, 256), np.float32)
    for ry in (0, 1):
        for rx in (0, 1):
            cls = ry * 2 + rx
            for ky in _class_taps(ry):
                for kx in _class_taps(rx):
                    di = 1 if ky == ry else 0
                    dj = 1 if kx == rx else 0
                    g = di * 2 + dj
                    w3c[32 * g:32 * (g + 1), 64 * cls:64 * (cls + 1)] = \
                        w3[:, :, ky, kx]
    return w3c


def _stack_w3_tm(w3):
    """TM: per-tap block-diagonal weights [128, 9*256]: rows (g', c),
    cols (tap, g'', o) = w3[c, o, ky, kx] iff g'' == g'."""
    w3b = np.zeros((128, 9 * 256), np.float32)
    for ky in range(3):
        for kx in range(3):
            tap = ky * 3 + kx
            for g in range(4):
                w3b[32 * g:32 * (g + 1),
                    tap * 256 + 64 * g:tap * 256 + 64 * (g + 1)] = \
                    w3[:, :, ky, kx]
    return w3b


def _prep_in_maps(hist_traj, w1, b1, w2, b2, w3, b3):
    h = np.where(hist_traj == -1.0, 0.0, hist_traj).astype(np.float32)
    xA = np.ascontiguousarray(h[0].T)  # [3, T]
    xB = np.ascontiguousarray(h[1].T)
    common = {
        "w1r": _stack_w1(w1),
        "w2r": _stack_w2(w2),
        "w3r": _stack_w3_tm(w3) if TM else _stack_w3(w3),
        "b1c": _pad_groups(b1, 16).reshape(128, 1),
        "b2c": np.ascontiguousarray(np.tile(b2.reshape(1, 32),
            (4, 1)).reshape(128, 1)).astype(np.float32),
        "b3c": (np.tile(np.tile(np.asarray(b3, np.float32), 4)[None, :],
                        (128, 1)) if TM else
                np.ascontiguousarray(b3.reshape(64, 1)).astype(np.float32)),
        "xb": xB,
    }
    in_maps = []
    for core in range(N_CORES):
        m = dict(common)
        m["xa"] = xA if core == 0 else xB
        in_maps.append(m)
    return in_maps


def run_on_hw(hist_traj, w1, b1, w2, b2, w3, b3, trace=False,
              trace_cores=None):
    """Returns ((enc_out, seq_len), BassKernelResults)."""
    from concourse.bass_utils import run_bass_kernel_spmd

    nc = _get_nc()
    in_maps = _prep_in_maps(hist_traj, w1, b1, w2, b2, w3, b3)
    res = run_bass_kernel_spmd(nc, in_maps, list(range(N_CORES)),
                               trace=trace, trace_cores=trace_cores)
    enc = np.concatenate([r["out"] for r in res.results], axis=0)
    enc = enc.reshape(B * T, 64, 16, 16)
    seq_len = np.full((B,), T, dtype=np.int32)
    return (enc, seq_len), res


def kernel(hist_traj, w1, b1, w2, b2, w3, b3):
    out, _ = run_on_hw(hist_traj, w1, b1, w2, b2, w3, b3)
    return out
